# revision 1
# baseline (speedup 1.0000x reference)
"""Trainium2 Bass kernel for nn_BPR_76665166234050 (3-hop LightGCN + BPR loss).

Strategy (8 NeuronCores, SPMD single program):
- Destinations sharded across cores; each core owns all edges into its shard,
  so per-hop segment sums are exact per core (no partial-sum all-reduce).
- Per hop: dma_gather source rows (int16 indices, tables chunked at 25088
  rows), build narrow one-hot*val lhsT matrices on DVE via broadcast-AP
  tensor_tensor ops, segment-sum on the PE into PSUM with 32-aligned window
  matmuls, drain to HBM.
- AllGather (ncfw) rebuilds full tables between dependent hops; hop order
  g1u,g1i,g2i,g2u,g3u,g3i lets every AllGather overlap the next hop's compute.
- Tail: BPR batch via quad/pair gathers + masks, self-distillation norms on
  the local shard; per-core partial sums are combined on the host.
"""
import sys
sys.path.insert(0, "/opt/trn_rl_repo")
import numpy as np


def _rup(x, m):
    return (x + m - 1) // m * m


class CFG:
    def __init__(self, user=100000, item=50000, d=64, e=3200000, b=16384,
                 ncores=8, chunk=25088, wwin=32):
        self.USER, self.ITEM, self.D, self.E, self.B = user, item, d, e, b
        self.NC = ncores
        self.CHUNK = chunk
        self.W = wwin
        self.UPAD = _rup(user, 128 * ncores)
        self.IPAD = _rup(item, 128 * ncores)
        # chunk must divide padded sizes evenly-ish; just cover with ceil
        self.UCH = (self.UPAD + chunk - 1) // chunk
        self.ICH = (self.IPAD + chunk - 1) // chunk
        self.USH = self.UPAD // ncores
        self.ISH = self.IPAD // ncores
        self.UBLK = self.USH // 128
        self.IBLK = self.ISH // 128
        self.BSH = b // ncores           # batch entries per core
        assert b % (16 * ncores) == 0


def _prep_direction(cfg, dst, src, val, sh_rows, nblk, nsch):
    """Token schedule for one direction (dest-sharded, per-core arrays).

    Returns (meta, per_core) where meta is static structure shared by all
    cores and per_core holds idx16/winval/windst arrays for each core.
    """
    NC, CHUNK, W = cfg.NC, cfg.CHUNK, cfg.W
    core = dst // sh_rows
    blk = (dst % sh_rows) // 128
    dloc = dst % 128
    sch = src // CHUNK
    sloc = (src % CHUNK).astype(np.int16)

    order = np.lexsort((dloc, sch, blk, core))
    core, blk, dloc, sch, sloc, val = (a[order] for a in
                                       (core, blk, dloc, sch, sloc, val))

    # counts per (core, blk, sch)
    key = (core * nblk + blk) * nsch + sch
    counts = np.bincount(key, minlength=NC * nblk * nsch).reshape(NC, nblk, nsch)
    caps = np.maximum(_rup(counts.max(axis=0), 128), 128)   # [nblk, nsch]

    offs = np.zeros((nblk, nsch), np.int64)
    t = 0
    for b in range(nblk):
        for c in range(nsch):
            offs[b, c] = t
            t += caps[b, c]
    T = t
    NCHT = T // 128

    # position of each edge in its core's token slab
    seg_sorted = key  # already sorted
    seg_start = np.zeros(NC * nblk * nsch + 1, np.int64)
    np.cumsum(np.bincount(seg_sorted, minlength=NC * nblk * nsch),
              out=seg_start[1:])
    rank = np.arange(len(dst)) - seg_start[seg_sorted]
    pos = offs[blk, sch] + rank

    idx_all = np.zeros((NC, T), np.int16)
    val_all = np.zeros((NC, T), np.float32)
    dloc_all = np.zeros((NC, T), np.int16)
    idx_all[core, pos] = sloc
    val_all[core, pos] = val
    dloc_all[core, pos] = dloc

    # per-K-chunk dest range across all cores
    cid = pos // 128
    lo = np.full(NCHT, 255, np.int64)
    hi = np.zeros(NCHT, np.int64)
    np.minimum.at(lo, cid, dloc)
    np.maximum.at(hi, cid, dloc)
    lo[lo == 255] = 0

    # windows per block: (local chunk idx, base); ensure 32-group coverage
    blocks = []
    nw = 0
    for b in range(nblk):
        c0 = offs[b, 0] // 128
        cend = (offs[b, 0] + caps[b].sum()) // 128
        # group-major window order: each 32-row psum group opens (start)
        # and closes (stop) before the next one, keeping accumulation
        # groups disjoint in time.
        wins, gstart, gstop = [], [], []
        for g in range(128 // W):
            glist = [cj - c0 for cj in range(c0, cend)
                     if lo[cj] // W <= g <= hi[cj] // W]
            if not glist:
                glist = [0]  # dummy coverage window (mask all-zero)
            for i, cj in enumerate(glist):
                wins.append((cj, g * W))
                gstart.append(i == 0)
                gstop.append(i == len(glist) - 1)
        blocks.append(dict(
            c0=c0, nch=cend - c0, woff=nw, wins=wins,
            gstart=gstart, gstop=gstop,
            goffs=[(int(offs[b, c]), int(caps[b, c])) for c in range(nsch)],
        ))
        nw += len(wins)

    # per-core device arrays
    per_core = []
    for k in range(NC):
        vw = val_all[k].reshape(NCHT, 128).T            # [128, NCHT]
        dw = dloc_all[k].astype(np.float32).reshape(NCHT, 128).T
        winval = np.zeros((128, nw), np.float32)
        windst = np.zeros((128, nw), np.float32)
        for bmeta in blocks:
            for w, (cj, base) in enumerate(bmeta["wins"]):
                gw = bmeta["woff"] + w
                winval[:, gw] = vw[:, bmeta["c0"] + cj]
                windst[:, gw] = dw[:, bmeta["c0"] + cj] - base
        iw = idx_all[k].reshape(T // 16, 16).T          # [16, T/16]
        idx16 = np.tile(iw, (8, 1))                     # [128, T/16]
        per_core.append(dict(idx=idx16, wval=winval, wdst=windst))

    meta = dict(T=T, NW=nw, blocks=blocks, caps=caps, offs=offs, nsch=nsch,
                nblk=nblk)
    return meta, per_core


def _wrap_shard(tbl_pad, k, sh_rows):
    """[sh_rows, D] shard -> [128, (sh_rows/128)*D] wrapped for SBUF."""
    s = tbl_pad[k * sh_rows:(k + 1) * sh_rows]
    nb = sh_rows // 128
    d = s.shape[1]
    return s.reshape(nb, 128, d).transpose(1, 0, 2).reshape(128, nb * d).copy()


def _wrap_vec(vec_pad, k, sh_rows):
    s = vec_pad[k * sh_rows:(k + 1) * sh_rows]
    nb = sh_rows // 128
    return s.reshape(nb, 128).T.copy()


def _wrap_idx(ix, n):
    """flat int indices -> [128, n/16] int16 gather layout."""
    w = ix.astype(np.int16).reshape(n // 16, 16).T
    return np.tile(w, (8, 1))


def build_program(cfg, mu, mi, nhops=6, with_ag=True, with_tail=True,
                  repeat=1):
    """Build the Bass/Tile program. mu/mi: direction metas (static)."""
    import concourse.bass as bass
    import concourse.bacc as bacc
    import concourse.tile as tile
    from concourse import mybir

    D, W, NC = cfg.D, cfg.W, cfg.NC
    f32, i16 = mybir.dt.float32, mybir.dt.int16
    AOT = mybir.AluOpType

    nc = bacc.Bacc("TRN2", target_bir_lowering=False, debug=False,
                   num_devices=NC)

    # ---- I/O ----
    uemb = nc.dram_tensor("uemb", [cfg.UPAD, D], f32, kind="ExternalInput")
    iemb = nc.dram_tensor("iemb", [cfg.IPAD, D], f32, kind="ExternalInput")
    uemb_sh = nc.dram_tensor("uemb_sh", [128, cfg.UBLK * D], f32, kind="ExternalInput")
    iemb_sh = nc.dram_tensor("iemb_sh", [128, cfg.IBLK * D], f32, kind="ExternalInput")
    oldu_sh = nc.dram_tensor("oldu_sh", [128, cfg.UBLK * D], f32, kind="ExternalInput")
    oldi_sh = nc.dram_tensor("oldi_sh", [128, cfg.IBLK * D], f32, kind="ExternalInput")
    nu_sh = nc.dram_tensor("nu_sh", [128, cfg.UBLK], f32, kind="ExternalInput")
    ni_sh = nc.dram_tensor("ni_sh", [128, cfg.IBLK], f32, kind="ExternalInput")
    idx_u = nc.dram_tensor("idx_u", [128, mu["T"] // 16], i16, kind="ExternalInput")
    idx_i = nc.dram_tensor("idx_i", [128, mi["T"] // 16], i16, kind="ExternalInput")
    wval_u = nc.dram_tensor("wval_u", [128, mu["NW"]], f32, kind="ExternalInput")
    wdst_u = nc.dram_tensor("wdst_u", [128, mu["NW"]], f32, kind="ExternalInput")
    wval_i = nc.dram_tensor("wval_i", [128, mi["NW"]], f32, kind="ExternalInput")
    wdst_i = nc.dram_tensor("wdst_i", [128, mi["NW"]], f32, kind="ExternalInput")
    iota_in = nc.dram_tensor("iota", [128, W], f32, kind="ExternalInput")
    BSH = cfg.BSH
    bidx_u = nc.dram_tensor("bidx_u", [128, BSH // 16], i16, kind="ExternalInput")
    bidx_i = nc.dram_tensor("bidx_i", [128, BSH // 16], i16, kind="ExternalInput")
    bidx_j = nc.dram_tensor("bidx_j", [128, BSH // 16], i16, kind="ExternalInput")
    # 8 masks: 4 user quarters, 2 item_i halves, 2 item_j halves
    bmask = nc.dram_tensor("bmask", [128, (BSH // 128) * 8], f32, kind="ExternalInput")
    ones_in = nc.dram_tensor("ones", [128, 1], f32, kind="ExternalInput")
    out_d = nc.dram_tensor("out", [4], f32, kind="ExternalOutput")

    # ---- internal DRAM: AG buffers ----
    def ag_pair(name, sh, full):
        if not with_ag:
            return None, None
        a = nc.dram_tensor(f"agin_{name}", [sh, D], f32, kind="Internal")
        o = nc.dram_tensor(f"agout_{name}", [full, D], f32, kind="Internal",
                           addr_space="Shared")
        return a, o

    agin_g1u, agout_g1u = ag_pair("g1u", cfg.USH, cfg.UPAD)
    agin_g1i, agout_g1i = ag_pair("g1i", cfg.ISH, cfg.IPAD)
    agin_g2i, agout_g2i = ag_pair("g2i", cfg.ISH, cfg.IPAD)
    agin_g2u, agout_g2u = ag_pair("g2u", cfg.USH, cfg.UPAD)
    agin_gcu, agout_gcu = ag_pair("gcu", cfg.USH, cfg.UPAD)
    agin_gci, agout_gci = ag_pair("gci", cfg.ISH, cfg.IPAD)

    maxT_u = max(sum(c for _, c in b["goffs"]) for b in mu["blocks"])
    maxT_i = max(sum(c for _, c in b["goffs"]) for b in mi["blocks"])
    maxT = max(maxT_u, maxT_i)
    maxW = max(max(len(b["wins"]) for b in mu["blocks"]),
               max(len(b["wins"]) for b in mi["blocks"]))

    with tile.TileContext(nc) as tc:
        with (
            tc.tile_pool(name="persist", bufs=1) as pp,
            tc.tile_pool(name="io", bufs=3) as iop,
            tc.tile_pool(name="gath", bufs=3) as gp,
            tc.tile_pool(name="lhs", bufs=3) as lp,
            tc.tile_pool(name="drain", bufs=4) as dp,
            tc.tile_pool(name="psum", bufs=4, space="PSUM") as psp,
            tc.tile_pool(name="tail", bufs=1) as tp,
        ):
            gcn_u = pp.tile([128, cfg.UBLK, D], f32, tag="gcn_u")
            gcn_i = pp.tile([128, cfg.IBLK, D], f32, tag="gcn_i")
            iota_t = pp.tile([128, W], f32, tag="iota")
            nc.sync.dma_start(iota_t[:], iota_in.ap())
            nc.sync.dma_start(gcn_u[:], uemb_sh.ap().rearrange(
                "p (b d) -> p b d", d=D))
            nc.sync.dma_start(gcn_i[:], iemb_sh.ap().rearrange(
                "p (b d) -> p b d", d=D))

            def hop(meta, idx_d, wval_d, wdst_d, src_buf, src_rows, wgt,
                    acc_tile, ag_in):
                """One spmm hop. src_buf: DRAM handle [src_rows, D]."""
                src_ap = src_buf.ap()
                for b, bm in enumerate(meta["blocks"]):
                    Tb = sum(c for _, c in bm["goffs"])
                    nwb = len(bm["wins"])
                    off0 = bm["goffs"][0][0]
                    idx_t = iop.tile([128, maxT // 16], i16, tag="idx")
                    wv_t = iop.tile([128, maxW], f32, tag="wv")
                    wd_t = iop.tile([128, maxW], f32, tag="wd")
                    nc.sync.dma_start(
                        idx_t[:, :Tb // 16],
                        idx_d.ap()[:, off0 // 16:(off0 + Tb) // 16])
                    nc.sync.dma_start(
                        wv_t[:, :nwb],
                        wval_d.ap()[:, bm["woff"]:bm["woff"] + nwb])
                    nc.sync.dma_start(
                        wd_t[:, :nwb],
                        wdst_d.ap()[:, bm["woff"]:bm["woff"] + nwb])

                    g_t = gp.tile([128, maxT // 128, D], f32, tag="g")
                    for c, (off, cap) in enumerate(bm["goffs"]):
                        if cap == 0:
                            continue
                        rel = off - off0
                        lo_row = c * cfg.CHUNK
                        hi_row = min(lo_row + cfg.CHUNK, src_rows)
                        nc.gpsimd.dma_gather(
                            g_t[:, rel // 128:(rel + cap) // 128, :],
                            src_ap[lo_row:hi_row, :],
                            idx_t[:, rel // 16:(rel + cap) // 16],
                            num_idxs=cap,
                            num_idxs_reg=cap,
                            elem_size=D,
                            single_packet=False,
                        )

                    l_t = lp.tile([128, maxW, W], f32, tag="l")
                    dst_b = wd_t[:, :nwb].broadcast_to([128, nwb, W])
                    iota_b = iota_t[:].rearrange(
                        "p (c w) -> p c w", c=1).broadcast_to([128, nwb, W])
                    val_b = wv_t[:, :nwb].broadcast_to([128, nwb, W])
                    nc.vector.tensor_tensor(
                        l_t[:, :nwb, :], dst_b, iota_b, AOT.is_equal)
                    nc.vector.tensor_tensor(
                        l_t[:, :nwb, :], l_t[:, :nwb, :], val_b, AOT.mult)

                    ps_t = psp.tile([128, D], f32, tag="ps")
                    for w, (cj, base) in enumerate(bm["wins"]):
                        nc.tensor.matmul(
                            ps_t[base:base + W, :],
                            l_t[:, w, :],
                            g_t[:, cj, :],
                            start=bm["gstart"][w],
                            stop=bm["gstop"][w],
                            tile_position=(0, base),
                        )

                    if ag_in is not None:
                        dr_t = dp.tile([128, D], f32, tag="dr")
                        nc.scalar.copy(dr_t[:], ps_t[:])
                        nc.sync.dma_start(
                            ag_in.ap()[b * 128:(b + 1) * 128, :], dr_t[:])
                    nc.vector.scalar_tensor_tensor(
                        acc_tile[:, b, :], ps_t[:], float(wgt),
                        acc_tile[:, b, :], AOT.mult, AOT.add)

            def allgather(ag_in, ag_out):
                if not with_ag:
                    return
                nc.gpsimd.collective_compute(
                    "AllGather", mybir.AluOpType.bypass,
                    replica_groups=[list(range(NC))],
                    ins=[ag_in.ap()], outs=[ag_out.ap()],
                )

            U = (mu, idx_u, wval_u, wdst_u)
            I = (mi, idx_i, wval_i, wdst_i)
            src_g1u = agout_g1u if with_ag else uemb
            src_g1i = agout_g1i if with_ag else iemb
            src_g2i = agout_g2i if with_ag else iemb
            src_g2u = agout_g2u if with_ag else uemb
            hops = [
                # g1u = A @ item_emb
                lambda: (hop(*U, iemb, cfg.IPAD, 0.5, gcn_u, agin_g1u),
                         allgather(agin_g1u, agout_g1u)),
                # g1i = A^T @ user_emb
                lambda: (hop(*I, uemb, cfg.UPAD, 0.5, gcn_i, agin_g1i),
                         allgather(agin_g1i, agout_g1i)),
                # g2i = A^T @ g1u
                lambda: (hop(*I, src_g1u, cfg.UPAD, 1.0 / 3.0, gcn_i,
                             agin_g2i),
                         allgather(agin_g2i, agout_g2i)),
                # g2u = A @ g1i
                lambda: (hop(*U, src_g1i, cfg.IPAD, 1.0 / 3.0, gcn_u,
                             agin_g2u),
                         allgather(agin_g2u, agout_g2u)),
                # g3u = A @ g2i
                lambda: (hop(*U, src_g2i, cfg.IPAD, 0.25, gcn_u, None),
                         with_ag and (nc.sync.dma_start(
                             agin_gcu.ap().rearrange("(b p) d -> p b d",
                                                     p=128), gcn_u[:]),
                             allgather(agin_gcu, agout_gcu))),
                # g3i = A^T @ g2u
                lambda: (hop(*I, src_g2u, cfg.UPAD, 0.25, gcn_i, None),
                         with_ag and (nc.sync.dma_start(
                             agin_gci.ap().rearrange("(b p) d -> p b d",
                                                     p=128), gcn_i[:]),
                             allgather(agin_gci, agout_gci))),
            ]
            for _rep in range(repeat):
                for h in hops[:nhops]:
                    h()

            # ---------------- tail ----------------
            part_t = tp.tile([128, 4], f32, tag="part")
            if not with_tail:
                nc.vector.memset(part_t[:], 0.0)

            # self-distillation partials from local shards
            def self_loss(acc_tile, old_d, n_d, nblk, col):
                old_t = tp.tile([128, nblk, D], f32, tag=f"old{col}")
                nv_t = tp.tile([128, nblk], f32, tag=f"nv{col}")
                nc.sync.dma_start(old_t[:], old_d.ap().rearrange(
                    "p (b d) -> p b d", d=D))
                nc.sync.dma_start(nv_t[:], n_d.ap())
                nc.vector.tensor_tensor(old_t[:], acc_tile[:], old_t[:],
                                        AOT.subtract)
                nc.vector.tensor_tensor(old_t[:], old_t[:], old_t[:],
                                        AOT.mult)
                rs = tp.tile([128, nblk], f32, tag=f"rs{col}")
                nc.vector.tensor_reduce(rs[:], old_t[:],
                                        mybir.AxisListType.X, AOT.add)
                nc.scalar.activation(rs[:], rs[:],
                                     mybir.ActivationFunctionType.Sqrt)
                nc.vector.tensor_tensor(rs[:], rs[:], nv_t[:], AOT.mult)
                nc.vector.tensor_reduce(part_t[:, col:col + 1], rs[:],
                                        mybir.AxisListType.X, AOT.add)

            if with_tail:
                self_loss(gcn_u, oldu_sh, nu_sh, cfg.UBLK, 2)
                self_loss(gcn_i, oldi_sh, ni_sh, cfg.IBLK, 3)

            if with_tail:
                # BPR batch: gathers from AG'd gcn tables
                BS = BSH // 128  # free-dim slots
                mask_t = tp.tile([128, 8 * BS], f32, tag="bmask")
                nc.sync.dma_start(mask_t[:], bmask.ap())

                def batch_rows(src_full, rows_full, group, bidx_d, mask_lo, ngrp,
                               tag):
                    """gather fused rows [128, BS, group*D]; mask-select -> [128,BS,D]"""
                    gt_full = tp.tile([128, BS * 4 * D], f32, tag="bgshare")
                    gt = gt_full[:, :BS * group * D].rearrange(
                        "p (s gd) -> p s gd", gd=group * D)
                    bix_t = tp.tile([128, BSH // 16], i16, tag=f"bx{tag}")
                    nc.sync.dma_start(bix_t[:], bidx_d.ap())
                    src2 = src_full.ap().rearrange("(a g) d -> a (g d)", g=group)
                    nc.gpsimd.dma_gather(
                        gt[:], src2, bix_t[:],
                        num_idxs=BSH, num_idxs_reg=BSH, elem_size=group * D,
                    single_packet=False)
                    rt = tp.tile([128, BS, D], f32, tag=f"br{tag}")
                    tmp = tp.tile([128, BS, D], f32, tag="btshare")
                    for q in range(ngrp):
                        m_b = mask_t[:, (mask_lo + q) * BS:(mask_lo + q + 1) * BS]\
                            .broadcast_to([128, BS, D])
                        dstt = rt if q == 0 else tmp
                        nc.vector.tensor_tensor(
                            dstt[:], gt[:, :, q * D:(q + 1) * D], m_b, AOT.mult)
                        if q > 0:
                            nc.vector.tensor_tensor(rt[:], rt[:], tmp[:], AOT.add)
                    return rt

                u_t = batch_rows(agout_gcu, cfg.UPAD, 4, bidx_u, 0, 4, "u")
                ii_t = batch_rows(agout_gci, cfg.IPAD, 2, bidx_i, 4, 2, "i")
                ij_t = batch_rows(agout_gci, cfg.IPAD, 2, bidx_j, 6, 2, "j")

                pr = tp.tile([128, BS, D], f32, tag="pr")
                pi = tp.tile([128, BS], f32, tag="pi")
                pj = tp.tile([128, BS], f32, tag="pj")
                nc.vector.tensor_tensor(pr[:], u_t[:], ii_t[:], AOT.mult)
                nc.vector.tensor_reduce(pi[:], pr[:], mybir.AxisListType.X, AOT.add)
                nc.vector.tensor_tensor(pr[:], u_t[:], ij_t[:], AOT.mult)
                nc.vector.tensor_reduce(pj[:], pr[:], mybir.AxisListType.X, AOT.add)
                nc.vector.tensor_tensor(pi[:], pi[:], pj[:], AOT.subtract)
                # -log_sigmoid(x) summed: part0 = sum(ln(sigmoid(x))), negated on host
                bt = tp.tile([128, BS], f32, tag="bt2")
                nc.scalar.activation(bt[:], pi[:],
                                     mybir.ActivationFunctionType.Sigmoid)
                nc.scalar.activation(bt[:], bt[:],
                                     mybir.ActivationFunctionType.Ln,
                                     accum_out=part_t[:, 0:1])

                # reg = sum over batch of rowsum(u^2+ii^2+ij^2)
                rg = tp.tile([128, BS], f32, tag="rg")
                rgt = tp.tile([128, BS], f32, tag="rgt")
                nc.vector.tensor_tensor(pr[:], u_t[:], u_t[:], AOT.mult)
                nc.vector.tensor_reduce(rg[:], pr[:], mybir.AxisListType.X, AOT.add)
                nc.vector.tensor_tensor(pr[:], ii_t[:], ii_t[:], AOT.mult)
                nc.vector.tensor_reduce(rgt[:], pr[:], mybir.AxisListType.X, AOT.add)
                nc.vector.tensor_tensor(rg[:], rg[:], rgt[:], AOT.add)
                nc.vector.tensor_tensor(pr[:], ij_t[:], ij_t[:], AOT.mult)
                nc.vector.tensor_reduce(rgt[:], pr[:], mybir.AxisListType.X, AOT.add)
                nc.vector.tensor_tensor(rg[:], rg[:], rgt[:], AOT.add)
                nc.vector.tensor_reduce(part_t[:, 1:2], rg[:],
                                        mybir.AxisListType.X, AOT.add)

            # cross-partition sum of the 4 partial columns via ones-matmul
            ones_t = tp.tile([128, 1], f32, tag="ones")
            nc.sync.dma_start(ones_t[:], ones_in.ap())
            ps4 = psp.tile([4, 1], f32, tag="ps4")
            nc.tensor.matmul(ps4[:], part_t[:], ones_t[:],
                             start=True, stop=True)
            out_t = tp.tile([4, 1], f32, tag="out4")
            nc.scalar.copy(out_t[:], ps4[:])
            nc.sync.dma_start(out_d.ap().rearrange("(a b) -> a b", b=1),
                              out_t[:])

    nc.compile()
    return nc


def _preprocess(cfg, inputs):
    """Host prep: returns (mu, mi, in_maps)."""
    user = np.asarray(inputs["user"]).astype(np.int64)
    item_i = np.asarray(inputs["item_i"]).astype(np.int64)
    item_j = np.asarray(inputs["item_j"]).astype(np.int64)
    edge_u = np.asarray(inputs["edge_u"]).astype(np.int64)
    edge_i = np.asarray(inputs["edge_i"]).astype(np.int64)
    edge_val = np.asarray(inputs["edge_val"]).astype(np.float32)
    user_emb = np.asarray(inputs["user_emb"]).astype(np.float32)
    item_emb = np.asarray(inputs["item_emb"]).astype(np.float32)
    old_U = np.asarray(inputs["old_U_emb"]).astype(np.float32)
    old_I = np.asarray(inputs["old_I_emb"]).astype(np.float32)
    n_U = np.asarray(inputs["n_U"]).astype(np.float32)
    n_I = np.asarray(inputs["n_I"]).astype(np.float32)

    D = cfg.D

    def pad_rows(a, n):
        out = np.zeros((n,) + a.shape[1:], a.dtype)
        out[:len(a)] = a
        return out

    uemb_p = pad_rows(user_emb, cfg.UPAD)
    iemb_p = pad_rows(item_emb, cfg.IPAD)
    oldu_p = pad_rows(old_U, cfg.UPAD)
    oldi_p = pad_rows(old_I, cfg.IPAD)
    nu_p = pad_rows(n_U, cfg.UPAD)
    ni_p = pad_rows(n_I, cfg.IPAD)

    mu, pc_u = _prep_direction(cfg, edge_u, edge_i, edge_val,
                               cfg.USH, cfg.UBLK, cfg.ICH)
    mi, pc_i = _prep_direction(cfg, edge_i, edge_u, edge_val,
                               cfg.ISH, cfg.IBLK, cfg.UCH)

    iota = np.broadcast_to(np.arange(cfg.W, dtype=np.float32),
                           (128, cfg.W)).copy()
    ones = np.ones((128, 1), np.float32)

    in_maps = []
    BSH, BS = cfg.BSH, cfg.BSH // 128
    for k in range(cfg.NC):
        bs = slice(k * BSH, (k + 1) * BSH)
        bu, bi, bj = user[bs], item_i[bs], item_j[bs]
        masks = np.zeros((128, 8 * BS), np.float32)
        for q in range(4):
            m = (bu % 4 == q).astype(np.float32).reshape(BS, 128).T
            masks[:, q * BS:(q + 1) * BS] = m
        for q in range(2):
            m = (bi % 2 == q).astype(np.float32).reshape(BS, 128).T
            masks[:, (4 + q) * BS:(5 + q) * BS] = m
            m = (bj % 2 == q).astype(np.float32).reshape(BS, 128).T
            masks[:, (6 + q) * BS:(7 + q) * BS] = m
        in_maps.append({
            "uemb": uemb_p, "iemb": iemb_p,
            "uemb_sh": _wrap_shard(uemb_p, k, cfg.USH),
            "iemb_sh": _wrap_shard(iemb_p, k, cfg.ISH),
            "oldu_sh": _wrap_shard(oldu_p, k, cfg.USH),
            "oldi_sh": _wrap_shard(oldi_p, k, cfg.ISH),
            "nu_sh": _wrap_vec(nu_p, k, cfg.USH),
            "ni_sh": _wrap_vec(ni_p, k, cfg.ISH),
            "idx_u": pc_u[k]["idx"], "wval_u": pc_u[k]["wval"],
            "wdst_u": pc_u[k]["wdst"],
            "idx_i": pc_i[k]["idx"], "wval_i": pc_i[k]["wval"],
            "wdst_i": pc_i[k]["wdst"],
            "iota": iota, "ones": ones,
            "bidx_u": _wrap_idx(bu // 4, BSH),
            "bidx_i": _wrap_idx(bi // 2, BSH),
            "bidx_j": _wrap_idx(bj // 2, BSH),
            "bmask": masks,
        })
    return mu, mi, in_maps


def run(cfg, inputs, trace=False, use_sim=False, **bkw):
    from concourse import bass_utils
    mu, mi, in_maps = _preprocess(cfg, inputs)
    nc = build_program(cfg, mu, mi, **bkw)
    if use_sim:
        from concourse.bass_interp import MultiCoreSim
        sim = MultiCoreSim(nc, num_cores=cfg.NC, trace=False)
        cores = [sim.cores[i] for i in sorted(sim.cores)]
        for k, core in enumerate(cores):
            for name, arr in in_maps[k].items():
                core.tensor(name)[:] = arr
        sim.simulate(check_with_hw=False)

        class R:
            results = [{"out": np.array(core.tensor("out"))}
                       for core in cores]
        res = R()
    else:
        res = bass_utils.run_bass_kernel_spmd(
            nc, in_maps, core_ids=list(range(cfg.NC)), trace=trace)
    parts = np.stack([res.results[k]["out"] for k in range(cfg.NC)])
    tot = parts.sum(axis=0)          # [ln_sig_sum, reg_sum, ud_sum, id_sum]
    loss_bpr = -tot[0] / cfg.B + 1e-4 * tot[1] / cfg.B
    loss_self = tot[2] / cfg.USER + tot[3] / cfg.ITEM
    out = np.array([loss_bpr, 100.0 * loss_self, 1.0, 1.0], np.float32)
    return out, res


def kernel(**inputs):
    cfg = CFG()
    out, _ = run(cfg, inputs)
    return out



# revision 2
# speedup vs baseline: 1.0175x; 1.0175x over previous
"""Trainium2 Bass kernel v2 for nn_BPR_76665166234050 (3-hop LightGCN + BPR).

Strategy (8 NeuronCores, SPMD):
- Destination-sharded spmm per hop; per-core exact segment sums (no psum AR).
- 4 gather passes instead of 6: rounds 2 and 3 fuse two hops each by
  interleaving the two source tables into 128-wide bf16 rows, so one
  256B-token gather feeds two PSUM accumulations.
    P1 (U): g1u = A @ i0              src [i0 | 0]      bf16 [IPAD,128]
    P2 (I): g1i = At @ u0, g2i = At @ g1u   src [u0 | g1u] bf16 [UPAD,128]
    P3 (U): g2u = A @ g1i, g3u = A @ g2i    src [g1i | g2i] bf16 [IPAD,128]
    P4 (I): g3i = At @ g2u            src [g2u | 0]     bf16 [UPAD,128]
- Per (block, src-chunk): dma_gather (int16 idx) of 256B bf16 rows; DVE
  builds 64-wide one-hot*val windows (bf16); PE does one LDW + 1-2 matmuls
  per 128-token chunk into a [128,64] f32 PSUM per block (two half-block
  psum groups, naturally time-disjoint because tokens sort by dloc).
- AllGather (bf16 interleaved tables) between passes; gcn accumulators stay
  f32 in SBUF; tail (BPR batch + self-distillation) as in v1.
"""
import sys
sys.path.insert(0, "/opt/trn_rl_repo")
import numpy as np
import ml_dtypes

BF16 = ml_dtypes.bfloat16


def _rup(x, m):
    return (x + m - 1) // m * m


class CFG:
    def __init__(self, user=100000, item=50000, d=64, e=3200000, b=16384,
                 ncores=8, chunk=25088):
        self.USER, self.ITEM, self.D, self.E, self.B = user, item, d, e, b
        self.NC = ncores
        self.CHUNK = chunk
        self.UPAD = _rup(user, 128 * ncores)
        self.IPAD = _rup(item, 128 * ncores)
        self.UCH = (self.UPAD + chunk - 1) // chunk
        self.ICH = (self.IPAD + chunk - 1) // chunk
        self.USH = self.UPAD // ncores
        self.ISH = self.IPAD // ncores
        self.UBLK = self.USH // 128
        self.IBLK = self.ISH // 128
        self.BSH = b // ncores
        assert b % (16 * ncores) == 0


def _prep_direction(cfg, dst, src, val, sh_rows, nblk, nsch):
    """Token schedule for one direction (dst-sharded).

    Tokens sorted by (core, blk, sch, dloc); slot (p=rank%128,
    chunk=rank//128) within each (blk,sch) group, caps uniform across cores.
    Windows: per chunk, the half-blocks (0-63 / 64-127) its tokens touch in
    ANY core; one-hot = is_eq(dloc-64h, iota64)*val masks naturally.

    Returns (meta, per_core):
      meta: blocks -> dict(goffs=[(choff,cap)]*nsch, wins=[(cj,half)],
            gstart/gstop flags, woff, nch)
      per_core: idx16 [128,T/16] i16, wdst/wval [128,NW] bf16
    """
    NC = cfg.NC
    core = dst // sh_rows
    blk = (dst % sh_rows) // 128
    dloc = dst % 128
    sch = src // cfg.CHUNK
    sloc = (src % cfg.CHUNK).astype(np.int16)

    order = np.lexsort((dloc, sch, blk, core))
    core, blk, dloc, sch, sloc, val = (a[order] for a in
                                       (core, blk, dloc, sch, sloc, val))

    key = (core * nblk + blk) * nsch + sch
    ncell = NC * nblk * nsch
    counts = np.bincount(key, minlength=ncell).reshape(NC, nblk, nsch)
    caps = np.maximum(_rup(counts.max(axis=0), 128), 128)   # [nblk, nsch]

    # chunk offsets (in tokens) per (blk, sch)
    offs = np.zeros((nblk, nsch), np.int64)
    t = 0
    for b in range(nblk):
        for c in range(nsch):
            offs[b, c] = t
            t += caps[b, c]
    T = t
    NCHT = T // 128

    seg_start = np.zeros(ncell + 1, np.int64)
    np.cumsum(np.bincount(key, minlength=ncell), out=seg_start[1:])
    rank = np.arange(len(dst)) - seg_start[key]
    pos = offs[blk, sch] + rank
    cid = pos // 128          # global chunk id within core slab
    p = pos % 128

    idx_all = np.full((NC, T), -1, np.int16)
    val_all = np.zeros((NC, T), np.float32)
    dloc_all = np.full((NC, T), -1, np.int16)
    idx_all[core, pos] = sloc
    val_all[core, pos] = val
    dloc_all[core, pos] = dloc

    # per-chunk half membership unioned over cores
    lo = np.full(NCHT, 127, np.int64)
    hi = np.zeros(NCHT, np.int64)
    np.minimum.at(lo, cid, dloc)
    np.maximum.at(hi, cid, dloc)
    has0 = lo < 64
    has1 = hi >= 64

    blocks = []
    nw = 0
    for b in range(nblk):
        c0 = offs[b, 0] // 128
        cend = (offs[b, 0] + caps[b].sum()) // 128
        wins, gstart, gstop = [], [], []
        for h in range(2):
            hs = [cj - c0 for cj in range(c0, cend)
                  if (has0[cj] if h == 0 else has1[cj])]
            if not hs:
                hs = [0]          # dummy: all-masked window still clears psum
            for i, cj in enumerate(hs):
                wins.append((cj, h))
                gstart.append(i == 0)
                gstop.append(i == len(hs) - 1)
        mins = counts.min(axis=0)
        blocks.append(dict(
            c0=c0, nch=cend - c0, woff=nw, wins=wins,
            gstart=gstart, gstop=gstop,
            goffs=[(int(offs[b, c]), int(caps[b, c])) for c in range(nsch)],
            glos=[int(mins[b, c]) // 128 for c in range(nsch)],
        ))
        nw += len(wins)

    # per-(blk,sch) true counts per core, emission order (blk-major, sch)
    cnts = counts.transpose(0, 1, 2).reshape(NC, nblk * nsch).astype(np.int32)

    per_core = []
    for k in range(NC):
        dw = dloc_all[k].reshape(NCHT, 128).T.astype(np.float32)  # [128,NCHT]
        vw = val_all[k].reshape(NCHT, 128).T
        wdst = np.zeros((128, nw), BF16)
        wval = np.zeros((128, nw), BF16)
        for bm in blocks:
            for w, (cj, h) in enumerate(bm["wins"]):
                gw = bm["woff"] + w
                wdst[:, gw] = (dw[:, bm["c0"] + cj] - 64 * h).astype(BF16)
                wval[:, gw] = vw[:, bm["c0"] + cj].astype(BF16)
        iw = idx_all[k].reshape(T // 16, 16).T
        idx16 = np.tile(iw, (8, 1))
        per_core.append(dict(idx=idx16, wdst=wdst, wval=wval,
                             cnt=cnts[k].reshape(1, -1)))

    meta = dict(T=T, NW=nw, blocks=blocks, nsch=nsch, nblk=nblk)
    return meta, per_core


def _wrap_shard(tbl_pad, k, sh_rows):
    s = tbl_pad[k * sh_rows:(k + 1) * sh_rows]
    nb = sh_rows // 128
    d = s.shape[1]
    return s.reshape(nb, 128, d).transpose(1, 0, 2).reshape(128, nb * d).copy()


def _wrap_vec(vec_pad, k, sh_rows):
    s = vec_pad[k * sh_rows:(k + 1) * sh_rows]
    nb = sh_rows // 128
    return s.reshape(nb, 128).T.copy()


def _wrap_idx(ix, n):
    w = ix.astype(np.int16).reshape(n // 16, 16).T
    return np.tile(w, (8, 1))


def build_program(cfg, mu, mi, with_tail=True):
    import concourse.bass as bass
    import concourse.bacc as bacc
    import concourse.tile as tile
    from concourse import mybir

    D, NC = cfg.D, cfg.NC
    f32, bf16, i16 = mybir.dt.float32, mybir.dt.bfloat16, mybir.dt.int16
    AOT = mybir.AluOpType

    nc = bacc.Bacc("TRN2", target_bir_lowering=False, debug=False,
                   num_devices=NC)

    # ---- I/O ----
    i0p = nc.dram_tensor("i0p", [cfg.IPAD, 128], bf16, kind="ExternalInput")
    u0h = nc.dram_tensor("u0h", [cfg.USH, 128], bf16,
                         kind="ExternalInput")   # u0 half prefilled
    uemb_sh = nc.dram_tensor("uemb_sh", [128, cfg.UBLK * D], f32, kind="ExternalInput")
    iemb_sh = nc.dram_tensor("iemb_sh", [128, cfg.IBLK * D], f32, kind="ExternalInput")
    oldu_sh = nc.dram_tensor("oldu_sh", [128, cfg.UBLK * D], f32, kind="ExternalInput")
    oldi_sh = nc.dram_tensor("oldi_sh", [128, cfg.IBLK * D], f32, kind="ExternalInput")
    nu_sh = nc.dram_tensor("nu_sh", [128, cfg.UBLK], f32, kind="ExternalInput")
    ni_sh = nc.dram_tensor("ni_sh", [128, cfg.IBLK], f32, kind="ExternalInput")
    idx_u = nc.dram_tensor("idx_u", [128, mu["T"] // 16], i16, kind="ExternalInput")
    idx_i = nc.dram_tensor("idx_i", [128, mi["T"] // 16], i16, kind="ExternalInput")
    wd_u = nc.dram_tensor("wd_u", [128, mu["NW"]], bf16, kind="ExternalInput")
    wv_u = nc.dram_tensor("wv_u", [128, mu["NW"]], bf16, kind="ExternalInput")
    wd_i = nc.dram_tensor("wd_i", [128, mi["NW"]], bf16, kind="ExternalInput")
    wv_i = nc.dram_tensor("wv_i", [128, mi["NW"]], bf16, kind="ExternalInput")
    iota_in = nc.dram_tensor("iota", [128, 64], bf16, kind="ExternalInput")
    i32 = mybir.dt.int32
    cnt_u = nc.dram_tensor("cnt_u", [1, mu["nblk"] * mu["nsch"]], i32,
                           kind="ExternalInput")
    cnt_i = nc.dram_tensor("cnt_i", [1, mi["nblk"] * mi["nsch"]], i32,
                           kind="ExternalInput")
    BSH = cfg.BSH
    bidx_u = nc.dram_tensor("bidx_u", [128, BSH // 16], i16, kind="ExternalInput")
    bidx_i = nc.dram_tensor("bidx_i", [128, BSH // 16], i16, kind="ExternalInput")
    bidx_j = nc.dram_tensor("bidx_j", [128, BSH // 16], i16, kind="ExternalInput")
    bmask = nc.dram_tensor("bmask", [128, (BSH // 128) * 8], f32, kind="ExternalInput")
    ones_in = nc.dram_tensor("ones", [128, 1], f32, kind="ExternalInput")
    out_d = nc.dram_tensor("out", [4], f32, kind="ExternalOutput")

    # ---- internal DRAM ----
    def internal(name, rows, cols, dt=bf16, shared=False):
        kw = {"addr_space": "Shared"} if shared else {}
        return nc.dram_tensor(name, [rows, cols], dt, kind="Internal", **kw)

    agin_ug1 = internal("agin_ug1", cfg.USH, 128)
    ug1_full = internal("ug1_full", cfg.UPAD, 128, shared=True)
    agin_gg = internal("agin_gg", cfg.ISH, 128)
    gg_full = internal("gg_full", cfg.IPAD, 128, shared=True)
    agin_g2u = internal("agin_g2u", cfg.USH, 128)
    g2u_full = internal("g2u_full", cfg.UPAD, 128, shared=True)
    agin_gcu = internal("agin_gcu", cfg.USH, D, f32)
    gcu_full = internal("gcu_full", cfg.UPAD, D, f32, shared=True)
    agin_gci = internal("agin_gci", cfg.ISH, D, f32)
    gci_full = internal("gci_full", cfg.IPAD, D, f32, shared=True)

    maxch = max(max(b["nch"] for b in mu["blocks"]),
                max(b["nch"] for b in mi["blocks"]))
    maxw = max(max(len(b["wins"]) for b in mu["blocks"]),
               max(len(b["wins"]) for b in mi["blocks"]))

    with tile.TileContext(nc) as tc:
        with (
            tc.tile_pool(name="persist", bufs=1) as pp,
            tc.tile_pool(name="io", bufs=3) as iop,
            tc.tile_pool(name="gath", bufs=3) as gp,
            tc.tile_pool(name="lhs", bufs=3) as lp,
            tc.tile_pool(name="drain", bufs=4) as dp,
            tc.tile_pool(name="psum", bufs=3, space="PSUM") as psp,
            tc.tile_pool(name="psumB", bufs=3, space="PSUM") as pspB,
            tc.tile_pool(name="psum4", bufs=1, space="PSUM") as psp4,
            tc.tile_pool(name="tail", bufs=1) as tp,
        ):
            gcn_u = pp.tile([128, cfg.UBLK, D], f32, tag="gcn_u")
            gcn_i = pp.tile([128, cfg.IBLK, D], f32, tag="gcn_i")
            iota_t = pp.tile([128, 64], bf16, tag="iota")
            cntu_t = pp.tile([1, mu["nblk"] * mu["nsch"]], i32, tag="cntu")
            cnti_t = pp.tile([1, mi["nblk"] * mi["nsch"]], i32, tag="cnti")
            nc.sync.dma_start(cntu_t[:], cnt_u.ap())
            nc.sync.dma_start(cnti_t[:], cnt_i.ap())
            cnt_reg = nc.gpsimd.alloc_register("cnt_reg")
            nc.sync.dma_start(iota_t[:], iota_in.ap())

            nc.sync.dma_start(agin_ug1.ap(), u0h.ap())
            nc.sync.dma_start(gcn_u[:], uemb_sh.ap().rearrange(
                "p (b d) -> p b d", d=D))
            nc.sync.dma_start(gcn_i[:], iemb_sh.ap().rearrange(
                "p (b d) -> p b d", d=D))

            def gat(g_t, src_ap, idx_t, bm, src_rows, cnt_t, b, nsch):
                """Emit gathers for one block: per src chunk."""
                off0 = bm["goffs"][0][0]
                for c, (off, cap) in enumerate(bm["goffs"]):
                    if cap == 0:
                        continue
                    rel = off - off0
                    lo_ch = bm["glos"][c]
                    if lo_ch * 128 < cap:
                        nc.vector.memset(
                            g_t[:, (rel + lo_ch * 128) // 128:
                                (rel + cap) // 128, :], 0.0)
                    lo_row = c * cfg.CHUNK
                    hi_row = min(lo_row + cfg.CHUNK, src_rows)
                    col = b * nsch + c
                    nc.gpsimd.reg_load(cnt_reg, cnt_t[0:1, col:col + 1])
                    nc.gpsimd.dma_gather(
                        g_t[:, rel // 128:(rel + cap) // 128, :],
                        src_ap[lo_row:hi_row, :],
                        idx_t[:, rel // 16:(rel + cap) // 16],
                        num_idxs=cap,
                        num_idxs_reg=cnt_reg,
                        elem_size=128,
                        single_packet=False,
                    )

            def hop_pass(meta, idx_d, wd_d, wv_d, cnt_t, src_buf, src_rows,
                         fused,
                         wgtA, accA, wgtB=None, accB=None,
                         tblA=None, tblA_col=None, tblB=None, tblB_col=None):
                """One gather pass. fused: two hops from row halves."""
                src_ap = src_buf.ap()
                for b, bm in enumerate(meta["blocks"]):
                    Tb = sum(c for _, c in bm["goffs"])
                    nwb = len(bm["wins"])
                    off0 = bm["goffs"][0][0]
                    idx_t = iop.tile([128, (maxch * 128) // 16], i16, tag="idx")
                    wdt = iop.tile([128, maxw], bf16, tag="wd")
                    wvt = iop.tile([128, maxw], bf16, tag="wv")
                    nc.sync.dma_start(
                        idx_t[:, :Tb // 16],
                        idx_d.ap()[:, off0 // 16:(off0 + Tb) // 16])
                    nc.sync.dma_start(
                        wdt[:, :nwb], wd_d.ap()[:, bm["woff"]:bm["woff"] + nwb])
                    nc.sync.dma_start(
                        wvt[:, :nwb], wv_d.ap()[:, bm["woff"]:bm["woff"] + nwb])

                    g_t = gp.tile([128, maxch, 128], bf16, tag="g")
                    gat(g_t, src_ap, idx_t, bm, src_rows, cnt_t, b,
                        meta["nsch"])

                    l_t = lp.tile([128, maxw, 64], bf16, tag="l")
                    dst_b = wdt[:, :nwb].broadcast_to([128, nwb, 64])
                    iota_b = iota_t[:].rearrange(
                        "p (c w) -> p c w", c=1).broadcast_to([128, nwb, 64])
                    val_b = wvt[:, :nwb].broadcast_to([128, nwb, 64])
                    nc.vector.tensor_tensor(
                        l_t[:, :nwb, :], dst_b, iota_b, AOT.is_equal)
                    nc.vector.tensor_tensor(
                        l_t[:, :nwb, :], l_t[:, :nwb, :], val_b, AOT.mult)

                    psA = psp.tile([128, D], f32, tag="psA")
                    if fused:
                        psB = pspB.tile([128, D], f32, tag="psB")
                    else:
                        psB = None
                    for w, (cj, h) in enumerate(bm["wins"]):
                        st, sp = bm["gstart"][w], bm["gstop"][w]
                        nc.tensor.matmul(
                            psA[64 * h:64 * h + 64, :],
                            l_t[:, w, :],
                            g_t[:, cj, 0:64],
                            start=st, stop=sp,
                            tile_position=(0, 64 * h),
                        )
                        if fused:
                            nc.tensor.matmul(
                                psB[64 * h:64 * h + 64, :],
                                l_t[:, w, :],
                                g_t[:, cj, 64:128],
                                start=st, stop=sp,
                                tile_position=(0, 64 * h),
                            )

                    def drain(ps, wgt, acc, tbl, tbl_col):
                        nc.vector.scalar_tensor_tensor(
                            acc[:, b, :], ps[:], float(wgt),
                            acc[:, b, :], AOT.mult, AOT.add)
                        if tbl is not None:
                            cv = dp.tile([128, D], bf16, tag="cv")
                            nc.scalar.copy(cv[:], ps[:])
                            nc.sync.dma_start(
                                tbl.ap()[b * 128:(b + 1) * 128,
                                         tbl_col:tbl_col + D],
                                cv[:])
                    drain(psA, wgtA, accA, tblA, tblA_col)
                    if fused:
                        drain(psB, wgtB, accB, tblB, tblB_col)

            def allgather(ag_in, ag_out):
                nc.gpsimd.collective_compute(
                    "AllGather", mybir.AluOpType.bypass,
                    replica_groups=[list(range(NC))],
                    ins=[ag_in.ap()], outs=[ag_out.ap()],
                )

            U = (mu, idx_u, wd_u, wv_u, cntu_t)
            I = (mi, idx_i, wd_i, wv_i, cnti_t)

            # P1: g1u = A @ i0
            hop_pass(*U, i0p, cfg.IPAD, False, 0.5, gcn_u,
                     tblA=agin_ug1, tblA_col=64)
            allgather(agin_ug1, ug1_full)
            # P2: g1i = At@u0 (cols 0:64), g2i = At@g1u (cols 64:128)
            hop_pass(*I, ug1_full, cfg.UPAD, True, 0.5, gcn_i, 1.0 / 3.0,
                     gcn_i, tblA=agin_gg, tblA_col=0,
                     tblB=agin_gg, tblB_col=64)
            allgather(agin_gg, gg_full)
            # P3: g2u = A@g1i, g3u = A@g2i (g3u drain fills unused col half)
            hop_pass(*U, gg_full, cfg.IPAD, True, 1.0 / 3.0, gcn_u, 0.25,
                     gcn_u, tblA=agin_g2u, tblA_col=0,
                     tblB=agin_g2u, tblB_col=64)
            allgather(agin_g2u, g2u_full)
            # gcn_u complete -> AG for tail (overlaps P4)
            nc.sync.dma_start(
                agin_gcu.ap().rearrange("(b p) d -> p b d", p=128), gcn_u[:])
            allgather(agin_gcu, gcu_full)
            # P4: g3i = At @ g2u
            hop_pass(*I, g2u_full, cfg.UPAD, False, 0.25, gcn_i)
            nc.sync.dma_start(
                agin_gci.ap().rearrange("(b p) d -> p b d", p=128), gcn_i[:])
            allgather(agin_gci, gci_full)

            # ---------------- tail ----------------
            part_t = tp.tile([128, 4], f32, tag="part")
            if not with_tail:
                nc.vector.memset(part_t[:], 0.0)

            def self_loss(acc_tile, old_d, n_d, nblk, col):
                old_t = tp.tile([128, nblk, D], f32, tag=f"old{col}")
                nv_t = tp.tile([128, nblk], f32, tag=f"nv{col}")
                nc.sync.dma_start(old_t[:], old_d.ap().rearrange(
                    "p (b d) -> p b d", d=D))
                nc.sync.dma_start(nv_t[:], n_d.ap())
                nc.vector.tensor_tensor(old_t[:], acc_tile[:], old_t[:],
                                        AOT.subtract)
                nc.vector.tensor_tensor(old_t[:], old_t[:], old_t[:],
                                        AOT.mult)
                rs = tp.tile([128, nblk], f32, tag=f"rs{col}")
                nc.vector.tensor_reduce(rs[:], old_t[:],
                                        mybir.AxisListType.X, AOT.add)
                nc.scalar.activation(rs[:], rs[:],
                                     mybir.ActivationFunctionType.Sqrt)
                nc.vector.tensor_tensor(rs[:], rs[:], nv_t[:], AOT.mult)
                nc.vector.tensor_reduce(part_t[:, col:col + 1], rs[:],
                                        mybir.AxisListType.X, AOT.add)

            if with_tail:
                self_loss(gcn_u, oldu_sh, nu_sh, cfg.UBLK, 2)
                self_loss(gcn_i, oldi_sh, ni_sh, cfg.IBLK, 3)

                BS = BSH // 128
                mask_t = tp.tile([128, 8 * BS], f32, tag="bmask")
                nc.sync.dma_start(mask_t[:], bmask.ap())

                def batch_rows(src_full, rows_full, group, bidx_d, mask_lo,
                               ngrp, tag):
                    gt_full = tp.tile([128, BS * 4 * D], f32, tag="bgshare")
                    gt = gt_full[:, :BS * group * D].rearrange(
                        "p (s gd) -> p s gd", gd=group * D)
                    bix_t = tp.tile([128, BSH // 16], i16, tag=f"bx{tag}")
                    nc.sync.dma_start(bix_t[:], bidx_d.ap())
                    src2 = src_full.ap().rearrange("(a g) d -> a (g d)",
                                                   g=group)
                    nc.gpsimd.dma_gather(
                        gt[:], src2, bix_t[:],
                        num_idxs=BSH, num_idxs_reg=BSH, elem_size=group * D,
                        single_packet=False)
                    rt = tp.tile([128, BS, D], f32, tag=f"br{tag}")
                    tmp = tp.tile([128, BS, D], f32, tag="btshare")
                    for q in range(ngrp):
                        m_b = mask_t[:, (mask_lo + q) * BS:
                                     (mask_lo + q + 1) * BS]\
                            .broadcast_to([128, BS, D])
                        dstt = rt if q == 0 else tmp
                        nc.vector.tensor_tensor(
                            dstt[:], gt[:, :, q * D:(q + 1) * D], m_b,
                            AOT.mult)
                        if q > 0:
                            nc.vector.tensor_tensor(rt[:], rt[:], tmp[:],
                                                    AOT.add)
                    return rt

                u_t = batch_rows(gcu_full, cfg.UPAD, 4, bidx_u, 0, 4, "u")
                ii_t = batch_rows(gci_full, cfg.IPAD, 2, bidx_i, 4, 2, "i")
                ij_t = batch_rows(gci_full, cfg.IPAD, 2, bidx_j, 6, 2, "j")

                pr = tp.tile([128, BS, D], f32, tag="pr")
                pi = tp.tile([128, BS], f32, tag="pi")
                pj = tp.tile([128, BS], f32, tag="pj")
                nc.vector.tensor_tensor(pr[:], u_t[:], ii_t[:], AOT.mult)
                nc.vector.tensor_reduce(pi[:], pr[:], mybir.AxisListType.X,
                                        AOT.add)
                nc.vector.tensor_tensor(pr[:], u_t[:], ij_t[:], AOT.mult)
                nc.vector.tensor_reduce(pj[:], pr[:], mybir.AxisListType.X,
                                        AOT.add)
                nc.vector.tensor_tensor(pi[:], pi[:], pj[:], AOT.subtract)
                bt = tp.tile([128, BS], f32, tag="bt2")
                nc.scalar.activation(bt[:], pi[:],
                                     mybir.ActivationFunctionType.Sigmoid)
                nc.scalar.activation(bt[:], bt[:],
                                     mybir.ActivationFunctionType.Ln,
                                     accum_out=part_t[:, 0:1])

                rg = tp.tile([128, BS], f32, tag="rg")
                rgt = tp.tile([128, BS], f32, tag="rgt")
                nc.vector.tensor_tensor(pr[:], u_t[:], u_t[:], AOT.mult)
                nc.vector.tensor_reduce(rg[:], pr[:], mybir.AxisListType.X,
                                        AOT.add)
                nc.vector.tensor_tensor(pr[:], ii_t[:], ii_t[:], AOT.mult)
                nc.vector.tensor_reduce(rgt[:], pr[:], mybir.AxisListType.X,
                                        AOT.add)
                nc.vector.tensor_tensor(rg[:], rg[:], rgt[:], AOT.add)
                nc.vector.tensor_tensor(pr[:], ij_t[:], ij_t[:], AOT.mult)
                nc.vector.tensor_reduce(rgt[:], pr[:], mybir.AxisListType.X,
                                        AOT.add)
                nc.vector.tensor_tensor(rg[:], rg[:], rgt[:], AOT.add)
                nc.vector.tensor_reduce(part_t[:, 1:2], rg[:],
                                        mybir.AxisListType.X, AOT.add)

            ones_t = tp.tile([128, 1], f32, tag="ones")
            nc.sync.dma_start(ones_t[:], ones_in.ap())
            ps4 = psp4.tile([4, 1], f32, tag="ps4")
            nc.tensor.matmul(ps4[:], part_t[:], ones_t[:],
                             start=True, stop=True)
            out_t = tp.tile([4, 1], f32, tag="out4")
            nc.scalar.copy(out_t[:], ps4[:])
            nc.sync.dma_start(out_d.ap().rearrange("(a b) -> a b", b=1),
                              out_t[:])

    nc.compile()
    return nc


def _preprocess(cfg, inputs):
    user = np.asarray(inputs["user"]).astype(np.int64)
    item_i = np.asarray(inputs["item_i"]).astype(np.int64)
    item_j = np.asarray(inputs["item_j"]).astype(np.int64)
    edge_u = np.asarray(inputs["edge_u"]).astype(np.int64)
    edge_i = np.asarray(inputs["edge_i"]).astype(np.int64)
    edge_val = np.asarray(inputs["edge_val"]).astype(np.float32)
    user_emb = np.asarray(inputs["user_emb"]).astype(np.float32)
    item_emb = np.asarray(inputs["item_emb"]).astype(np.float32)
    old_U = np.asarray(inputs["old_U_emb"]).astype(np.float32)
    old_I = np.asarray(inputs["old_I_emb"]).astype(np.float32)
    n_U = np.asarray(inputs["n_U"]).astype(np.float32)
    n_I = np.asarray(inputs["n_I"]).astype(np.float32)

    D = cfg.D

    def pad_rows(a, n):
        out = np.zeros((n,) + a.shape[1:], a.dtype)
        out[:len(a)] = a
        return out

    uemb_p = pad_rows(user_emb, cfg.UPAD)
    iemb_p = pad_rows(item_emb, cfg.IPAD)
    oldu_p = pad_rows(old_U, cfg.UPAD)
    oldi_p = pad_rows(old_I, cfg.IPAD)
    nu_p = pad_rows(n_U, cfg.UPAD)
    ni_p = pad_rows(n_I, cfg.IPAD)

    # U direction: dst=user, src=item ; I direction: dst=item, src=user
    mu, pc_u = _prep_direction(cfg, edge_u, edge_i, edge_val,
                               cfg.USH, cfg.UBLK, cfg.ICH)
    mi, pc_i = _prep_direction(cfg, edge_i, edge_u, edge_val,
                               cfg.ISH, cfg.IBLK, cfg.UCH)

    i0p = np.zeros((cfg.IPAD, 128), BF16)
    i0p[:, :D] = iemb_p.astype(BF16)

    iota = np.broadcast_to(np.arange(64, dtype=np.float32),
                           (128, 64)).astype(BF16).copy()
    ones = np.ones((128, 1), np.float32)

    in_maps = []
    BSH, BS = cfg.BSH, cfg.BSH // 128
    for k in range(cfg.NC):
        u0h = np.zeros((cfg.USH, 128), BF16)
        u0h[:, :D] = uemb_p[k * cfg.USH:(k + 1) * cfg.USH].astype(BF16)
        bs = slice(k * BSH, (k + 1) * BSH)
        bu, bi, bj = user[bs], item_i[bs], item_j[bs]
        masks = np.zeros((128, 8 * BS), np.float32)
        for q in range(4):
            m = (bu % 4 == q).astype(np.float32).reshape(BS, 128).T
            masks[:, q * BS:(q + 1) * BS] = m
        for q in range(2):
            m = (bi % 2 == q).astype(np.float32).reshape(BS, 128).T
            masks[:, (4 + q) * BS:(5 + q) * BS] = m
            m = (bj % 2 == q).astype(np.float32).reshape(BS, 128).T
            masks[:, (6 + q) * BS:(7 + q) * BS] = m
        in_maps.append({
            "i0p": i0p, "u0h": u0h,
            "uemb_sh": _wrap_shard(uemb_p, k, cfg.USH),
            "iemb_sh": _wrap_shard(iemb_p, k, cfg.ISH),
            "oldu_sh": _wrap_shard(oldu_p, k, cfg.USH),
            "oldi_sh": _wrap_shard(oldi_p, k, cfg.ISH),
            "nu_sh": _wrap_vec(nu_p, k, cfg.USH),
            "ni_sh": _wrap_vec(ni_p, k, cfg.ISH),
            "idx_u": pc_u[k]["idx"], "wd_u": pc_u[k]["wdst"],
            "wv_u": pc_u[k]["wval"], "cnt_u": pc_u[k]["cnt"],
            "idx_i": pc_i[k]["idx"], "wd_i": pc_i[k]["wdst"],
            "wv_i": pc_i[k]["wval"], "cnt_i": pc_i[k]["cnt"],
            "iota": iota, "ones": ones,
            "bidx_u": _wrap_idx(bu // 4, BSH),
            "bidx_i": _wrap_idx(bi // 2, BSH),
            "bidx_j": _wrap_idx(bj // 2, BSH),
            "bmask": masks,
        })
    return mu, mi, in_maps


def run(cfg, inputs, trace=False, use_sim=False, **bkw):
    from concourse import bass_utils
    mu, mi, in_maps = _preprocess(cfg, inputs)
    nc = build_program(cfg, mu, mi, **bkw)
    if use_sim:
        from concourse.bass_interp import MultiCoreSim
        sim = MultiCoreSim(nc, num_cores=cfg.NC, trace=False)
        cores = [sim.cores[i] for i in sorted(sim.cores)]
        for k, core in enumerate(cores):
            for name, arr in in_maps[k].items():
                core.tensor(name)[:] = arr
        sim.simulate(check_with_hw=False)

        class R:
            results = [{"out": np.array(core.tensor("out"))}
                       for core in cores]
            exec_time_ns = None
        res = R()
    else:
        res = bass_utils.run_bass_kernel_spmd(
            nc, in_maps, core_ids=list(range(cfg.NC)), trace=trace)
    parts = np.stack([res.results[k]["out"] for k in range(cfg.NC)])
    tot = parts.sum(axis=0)
    loss_bpr = -tot[0] / cfg.B + 1e-4 * tot[1] / cfg.B
    loss_self = tot[2] / cfg.USER + tot[3] / cfg.ITEM
    out = np.array([loss_bpr, 100.0 * loss_self, 1.0, 1.0], np.float32)
    return out, res


def kernel(**inputs):
    cfg = CFG()
    out, _ = run(cfg, inputs)
    return out
